# revision 1
# baseline (speedup 1.0000x reference)
"""Trainium2 Bass kernel for nn_AuxLoss (aux CE loss + erf regularizer, segment-
mean over K=10 classes), data-parallel over 8 NeuronCores.

Math (per reference):
  f(u)      = 0.5 - 0.5*erf((-0.5 - u)/(sigma*sqrt2)) = 0.5 + 0.5*erf((u+0.5)*sqrt2)
  row_reg_n = sum_d f(u[n,d])
  row_ce_n  = logsumexp(yg[n,:]) - yg[n, yhat[n]]
  per-class means over rows with yhat==k, averaged over present classes:
  out = mean_k(seg_ce/cnt) + lmbd * mean_k(seg_reg/(cnt*D))

Device strategy per core (131072 rows):
  - partition p holds a contiguous slab of 1024 rows -> fully contiguous DMAs
    (u on the sync HWDGE ring, yg/yhat on the gpsimd SWDGE ring)
  - 16 chunks of w=64 rows/partition; ACT functions are batched over groups of
    chunks (erf xG, exp xG, ln xG, tapered at the end) and explicitly
    order-chained to amortize ACT table-set loads; tables prewarmed at t~0
  - per chunk one combined bf16 "work" tile [128, w, 75]:
      cols 0:64  erf(sqrt2*u + sqrt2/2)        (ACT, strided out; the 0.5+0.5*
                 affine is folded into the final fixup: seg_f = 0.5*D*cnt + 0.5*seg_erf)
      cols 64:74 onehot*yg                      (DVE; column-sum of this block's
                 segment-matmul = seg of yg[n,yhat[n]] -- diagonal trick)
      col  74    ones                           (counts)
    onehot[p,r,c] = (yhat==c) via iota compare (DVE, bf16); exp runs in-place
    on the yg tile; lse = ln(sumexp) feeds a DVE class-major masked reduce into
    an SBUF accumulator (so the PE stream never waits on ln) finished by one
    ones-stationary matmul
  - PE: per 128-row group g one matmul: onehot[:,g,:] stationary (128x10),
    work[:,g,:] moving (128x75), accumulating PSUM [10,75] over all 1024 groups
  - local pre-reduce to [10,4] (erf_sum, picked_sum, count, lse_sum), 160 B
    AllGather across 8 cores + local sum (a warm-up collective at t~0 absorbs
    the collective-stream entry barrier; its readback is pinned to the stream
    end to avoid DMA-queue head-of-line blocking), final means on-device.
"""

import math
import sys

if "/opt/trn_rl_repo" not in sys.path:
    sys.path.insert(0, "/opt/trn_rl_repo")

import numpy as np

N_CORES = 8
N_FULL = 1048576
C = 10
D = 64
P = 128
ROWS_PER_CORE = N_FULL // N_CORES  # 131072
SQ2 = math.sqrt(2.0)
W_COLS = D + C + 1  # erf block | onehot*yg block | ones


def build(rows_per_core=ROWS_PER_CORE, w=64, act_batch=4):
    """Build + compile the 8-core Bacc graph. w = rows/partition/chunk."""
    from concourse import bacc, mybir, tile

    f32 = mybir.dt.float32
    bf16 = mybir.dt.bfloat16
    i32 = mybir.dt.int32
    FT = mybir.ActivationFunctionType
    ALU = mybir.AluOpType
    AX = mybir.AxisListType

    rpp = rows_per_core // P
    assert rpp * P == rows_per_core
    nch = rpp // w
    assert nch * w == rpp
    sched = []
    rem = nch
    while rem > 0:
        step = act_batch if rem > 2 * act_batch else max(rem // 2, 1)
        step = min(step, rem)
        sched.append(step)
        rem -= step
    starts = [sum(sched[:i]) for i in range(len(sched))]
    nbatch = len(sched)

    nc = bacc.Bacc("TRN2", target_bir_lowering=False, debug=False, num_devices=N_CORES)

    yh_d = nc.dram_tensor("yhat", [rows_per_core], i32, kind="ExternalInput")
    yg_d = nc.dram_tensor("yg", [rows_per_core, C], f32, kind="ExternalInput")
    u_d = nc.dram_tensor("u_zg", [rows_per_core, D], f32, kind="ExternalInput")
    lm_d = nc.dram_tensor("lmbd", [1, 1], f32, kind="ExternalInput")
    out_d = nc.dram_tensor("out", [1, 1], f32, kind="ExternalOutput")
    warm_in = nc.dram_tensor("warm_in", [1, 1], f32)
    warm_out = nc.dram_tensor("warm_out", [1, 1], f32, addr_space="Shared")
    cc_in = nc.dram_tensor("cc_in", [C, 4], f32)
    cc_out = nc.dram_tensor("cc_out", [N_CORES, C, 4], f32, addr_space="Shared")

    u_v = u_d[:].rearrange("(p r) d -> p r d", p=P)
    yg_v = yg_d[:].rearrange("(p r) c -> p r c", p=P)
    yh_v = yh_d[:].rearrange("(p r) -> p r", p=P)

    rg = [list(range(N_CORES))]

    from concourse.tile_rust import add_dep_helper

    last_act = [None]

    def act_ordered(*args, **kwargs):
        """scalar.activation with an explicit chain dep so the Tile scheduler
        cannot interleave ACT functions (each interleave costs a ~1.3 us
        ACT table-set load)."""
        inst = nc.scalar.activation(*args, **kwargs)
        raw = getattr(inst, "ins", inst)
        if last_act[0] is not None:
            add_dep_helper(raw, last_act[0], sync=True, reason="act set order")
        last_act[0] = raw
        return inst

    with tile.TileContext(nc) as tc:
        with (
            tc.tile_pool(name="const", bufs=1) as constp,
            tc.tile_pool(name="io", bufs=1) as iop,
            tc.tile_pool(name="work", bufs=1) as workp,
            tc.tile_pool(name="psum", bufs=1, space="PSUM") as psump,
            tc.tile_pool(name="fin", bufs=1) as finp,
        ):
            # --- warm-up collective: absorb entry barrier early, overlapped
            # with the main DMA/compute stream ---
            wz = constp.tile([1, 1], f32)
            nc.vector.memset(wz[:], 0.0)
            nc.gpsimd.dma_start(warm_in[:], wz[:])
            nc.gpsimd.collective_compute(
                "AllReduce", ALU.add, replica_groups=rg,
                ins=[warm_in[:].opt()], outs=[warm_out[:].opt()],
            )
            wres = constp.tile([1, 1], f32)

            # --- constants ---
            erf_bias = constp.tile([P, 1], f32)
            nc.vector.memset(erf_bias[:], 0.5 * SQ2)
            iota_f = constp.tile([P, 1, C], f32)
            nc.gpsimd.iota(
                iota_f[:, 0, :], [[1, C]],
                channel_multiplier=0, allow_small_or_imprecise_dtypes=True,
            )
            yh_i = constp.tile([P, rpp], i32)
            nc.gpsimd.dma_start(yh_i[:], yh_v)
            yh_f = constp.tile([P, rpp], f32)
            nc.vector.tensor_copy(yh_f[:], yh_i[:])

            # prewarm the three ACT table sets while the first DMAs fly
            warm_act = constp.tile([1, 1], f32)
            nc.vector.memset(warm_act[:], 1.0)
            wa_o = constp.tile([1, 1], f32)
            act_ordered(wa_o[:], warm_act[:], FT.Ln)
            act_ordered(wa_o[:], warm_act[:], FT.Exp)
            act_ordered(wa_o[:], warm_act[:], FT.Erf, bias=erf_bias[0:1, :], scale=SQ2)

            ones128 = constp.tile([P, 1], f32)
            nc.vector.memset(ones128[:], 1.0)
            lacc = constp.tile([P, C], f32)
            nc.vector.memset(lacc[:], 0.0)

            ps = psump.tile([C, W_COLS], f32)
            ps_l = psump.tile([C, 1], f32)

            def chunks_of(b):
                return range(starts[b], starts[b] + sched[b])

            u_ts, yg_ts, work_ts, oh_ts, sume_ts = {}, {}, {}, {}, {}

            for b in range(nbatch):
                # DMAs: yg on the gpsimd (SWDGE) ring; u in 2-chunk 4 MB pairs
                # alternating between the sync (HWDGE) and gpsimd (SWDGE) paths
                for ci in chunks_of(b):
                    r0, r1 = ci * w, (ci + 1) * w
                    yg_t = iop.tile([P, w, C], f32, name="yg_t", bufs=2 * act_batch)
                    nc.gpsimd.dma_start(yg_t[:], yg_v[:, r0:r1, :])
                    yg_ts[ci] = yg_t
                    u_t = iop.tile([P, w, D], f32, name="u_t", bufs=5)
                    nc.sync.dma_start(u_t[:], u_v[:, r0:r1, :])
                    u_ts[ci] = u_t[:]

                # erf batch (one ACT table load)
                for ci in chunks_of(b):
                    work_t = workp.tile(
                        [P, w, W_COLS], bf16, name="work_t", bufs=2 * act_batch
                    )
                    act_ordered(
                        work_t[:, :, 0:D], u_ts.pop(ci), FT.Erf,
                        bias=erf_bias[:], scale=SQ2,
                    )
                    work_ts[ci] = work_t

                # DVE: onehot + onehot*yg + ones while ACT works
                for ci in chunks_of(b):
                    r0, r1 = ci * w, (ci + 1) * w
                    oh_t = workp.tile([P, w, C], bf16, name="oh_t", bufs=2 * act_batch)
                    nc.vector.tensor_tensor(
                        oh_t[:],
                        yh_f[:, r0:r1].broadcast_to([P, w, C]),
                        iota_f[:].broadcast_to([P, w, C]),
                        ALU.is_equal,
                    )
                    oh_ts[ci] = oh_t
                    work_t = work_ts[ci]
                    nc.vector.tensor_tensor(
                        work_t[:, :, D : D + C], oh_t[:], yg_ts[ci][:], ALU.mult
                    )
                    nc.vector.memset(work_t[:, :, D + C], 1.0)

                # PE: one matmul per 128-row group (gated only by erf + DVE)
                for ci in chunks_of(b):
                    work_t = work_ts.pop(ci)
                    oh_t = oh_ts[ci]
                    for g in range(w):
                        first = ci == 0 and g == 0
                        last = ci == nch - 1 and g == w - 1
                        nc.tensor.matmul(
                            ps[:], oh_t[:, g, :], work_t[:, g, :],
                            start=first, stop=last,
                        )

                # exp batch (one load), in-place into yg; row-sums on DVE
                for ci in chunks_of(b):
                    yg_t = yg_ts.pop(ci)
                    act_ordered(yg_t[:], yg_t[:], FT.Exp)
                    sume_t = workp.tile([P, w], f32, name="sume_t", bufs=act_batch + 1)
                    nc.vector.reduce_sum(sume_t[:], yg_t[:], axis=AX.X)
                    sume_ts[ci] = sume_t

                # ln batch (one load); lse segment-sum via DVE (class-major
                # masked blocked reduce) so the PE stream never waits on ln
                for ci in chunks_of(b):
                    lse_t = workp.tile([P, w], f32, name="lse_t", bufs=3)
                    act_ordered(lse_t[:], sume_ts.pop(ci)[:], FT.Ln)
                    oh_t = oh_ts.pop(ci)
                    oh_cm = oh_t[:].transpose([0, 2, 1])
                    ltmp = workp.tile([P, C, w], bf16, name="ltmp", bufs=2)
                    nc.vector.tensor_tensor(
                        ltmp[:], oh_cm,
                        lse_t[:].broadcast_to([P, w, C]).transpose([0, 2, 1]),
                        ALU.mult,
                    )
                    lred = workp.tile([P, C], f32, name="lred", bufs=2)
                    nc.vector.reduce_sum(lred[:], ltmp[:], axis=AX.X)
                    nc.vector.tensor_tensor(lacc[:], lacc[:], lred[:], ALU.add)

            # partition-reduce the lse accumulator: lacc.T @ ones lands the
            # [10,1] result on 10 partitions, matching the psum layout
            nc.tensor.matmul(ps_l[:], lacc[:], ones128[:], start=True, stop=True)

            # --- local pre-reduce -> [10, 4] (erf, picked, counts, lse) ---
            acc = finp.tile([C, 4], f32)
            nc.vector.reduce_sum(acc[:, 0:1], ps[:, 0:D], axis=AX.X)
            nc.vector.reduce_sum(acc[:, 1:2], ps[:, D : D + C], axis=AX.X)
            nc.vector.tensor_copy(acc[:, 2:3], ps[:, D + C : D + C + 1])
            nc.vector.tensor_copy(acc[:, 3:4], ps_l[:])
            nc.sync.dma_start(cc_in[:], acc[:])
            nc.gpsimd.collective_compute(
                "AllGather", ALU.bypass, replica_groups=rg,
                ins=[cc_in[:].opt()], outs=[cc_out[:].opt()],
            )
            # warm-up readback issued HERE (end of stream) with an explicit dep
            # so the scheduler cannot hoist it: if it lands early in the gpsimd
            # FIFO, its wait on the warm collective head-of-line-blocks every
            # yg DMA behind it for ~60 us.
            wres_dma = nc.gpsimd.dma_start(wres[:], warm_out[:])
            add_dep_helper(
                getattr(wres_dma, "ins", wres_dma), last_act[0],
                sync=True, reason="keep warm-up readback at stream end",
            )

            # --- gather-sum + final per-class means on partition 0 ---
            fing = finp.tile([1, N_CORES, C * 4], f32)
            nc.sync.dma_start(
                fing[:],
                cc_out[:].rearrange("(o n) k c -> o n (k c)", o=1),
            )
            finv = finp.tile([1, C * 4], f32)
            nc.vector.tensor_tensor(
                finv[:], fing[:, 0, :], fing[:, 1, :], ALU.add
            )
            for n in range(2, N_CORES):
                nc.vector.tensor_tensor(
                    finv[:], finv[:], fing[:, n, :], ALU.add
                )
            f3 = finv[:].rearrange("p (k c) -> p k c", k=C)  # [1, 10, 4]
            e10 = f3[:, :, 0]  # strided [1, 10] APs
            p10 = f3[:, :, 1]
            cn10 = f3[:, :, 2]
            l10 = f3[:, :, 3]

            ce10 = finp.tile([1, C], f32)  # seg_ce = seg_lse - seg_picked
            nc.vector.tensor_tensor(ce10[:], l10, p10, ALU.subtract)
            mask = finp.tile([1, C], f32)
            nc.vector.tensor_scalar(mask[:], cn10, 0.0, None, ALU.is_gt)
            one_m = finp.tile([1, C], f32)
            nc.vector.tensor_scalar(one_m[:], mask[:], -1.0, 1.0, ALU.mult, ALU.add)
            den = finp.tile([1, C], f32)
            nc.vector.tensor_tensor(den[:], one_m[:], cn10, ALU.add)
            rinv = finp.tile([1, C], f32)
            nc.vector.reciprocal(rinv[:], den[:])

            # reg_c = (0.5*cnt + seg_erf/(2D)) / cnt ; aux_c = seg_ce / cnt
            hc = finp.tile([1, C], f32)
            nc.vector.tensor_scalar(hc[:], cn10, 0.5, None, ALU.mult)
            rnum = finp.tile([1, C], f32)
            nc.vector.scalar_tensor_tensor(
                rnum[:], e10, 1.0 / (2.0 * D), hc[:], ALU.mult, ALU.add
            )
            regc = finp.tile([1, C], f32)
            nc.vector.tensor_mul(regc[:], rnum[:], rinv[:])
            auxc = finp.tile([1, C], f32)
            nc.vector.tensor_mul(auxc[:], ce10[:], rinv[:])

            nuq = finp.tile([1, 1], f32)
            nc.vector.reduce_sum(nuq[:], mask[:], axis=AX.X)
            ninv = finp.tile([1, 1], f32)
            nc.vector.reciprocal(ninv[:], nuq[:])
            sreg = finp.tile([1, 1], f32)
            nc.vector.reduce_sum(sreg[:], regc[:], axis=AX.X)
            saux = finp.tile([1, 1], f32)
            nc.vector.reduce_sum(saux[:], auxc[:], axis=AX.X)

            lm_t = finp.tile([1, 1], f32)
            nc.sync.dma_start(lm_t[:], lm_d[:])
            t1 = finp.tile([1, 1], f32)
            nc.vector.tensor_mul(t1[:], sreg[:], lm_t[:])
            t2 = finp.tile([1, 1], f32)
            nc.vector.tensor_add(t2[:], t1[:], saux[:])
            t3 = finp.tile([1, 1], f32)
            nc.vector.tensor_mul(t3[:], t2[:], ninv[:])
            # fold in 0*warm so the warm-up collective isn't dead code
            res = finp.tile([1, 1], f32)
            nc.vector.scalar_tensor_tensor(
                res[:], wres[:], 0.0, t3[:], ALU.mult, ALU.add
            )
            nc.sync.dma_start(out_d[:], res[:])

    nc.compile()
    return nc


_NC_CACHE = {}


def _get_nc(rows_per_core=ROWS_PER_CORE, w=64, act_batch=4):
    key = (rows_per_core, w, act_batch)
    if key not in _NC_CACHE:
        _NC_CACHE[key] = build(rows_per_core, w, act_batch)
    return _NC_CACHE[key]


def make_in_maps(yhat, yg, u_zg, lmbd, rows_per_core=ROWS_PER_CORE):
    yhat = np.ascontiguousarray(np.asarray(yhat).astype(np.int32))
    yg = np.ascontiguousarray(np.asarray(yg, dtype=np.float32))
    u_zg = np.ascontiguousarray(np.asarray(u_zg, dtype=np.float32))
    lmbd = np.asarray(lmbd, dtype=np.float32).reshape(1, 1)
    n = yhat.shape[0]
    assert n == rows_per_core * N_CORES
    in_maps = []
    for i in range(N_CORES):
        s = slice(i * rows_per_core, (i + 1) * rows_per_core)
        in_maps.append(
            {"yhat": yhat[s], "yg": yg[s], "u_zg": u_zg[s], "lmbd": lmbd}
        )
    return in_maps


def run(yhat, yg, u_zg, lmbd, trace=False, rows_per_core=ROWS_PER_CORE, w=64,
        act_batch=4):
    from concourse import bass_utils

    nc = _get_nc(rows_per_core, w, act_batch)
    in_maps = make_in_maps(yhat, yg, u_zg, lmbd, rows_per_core)
    res = bass_utils.run_bass_kernel_spmd(
        nc, in_maps, core_ids=list(range(N_CORES)), trace=trace
    )
    val = np.float32(np.asarray(res.results[0]["out"]).reshape(())[()])
    return val, res


def kernel(yhat, yg, u_zg, lmbd):
    val, _ = run(yhat, yg, u_zg, lmbd)
    return np.asarray(val, dtype=np.float32).reshape(())



# revision 2
# speedup vs baseline: 1.2476x; 1.2476x over previous
"""Trainium2 Bass kernel for nn_AuxLoss (aux CE loss + erf regularizer, segment-
mean over K=10 classes), data-parallel over 8 NeuronCores.

Math (per reference):
  f(u)      = 0.5 - 0.5*erf((-0.5 - u)/(sigma*sqrt2)) = 0.5 + 0.5*erf(sqrt2*u + sqrt2/2)
  row_reg_n = sum_d f(u[n,d])
  row_ce_n  = logsumexp(yg[n,:]) - yg[n, yhat[n]]
  per-class means over rows with yhat==k, averaged over present classes:
  out = mean_k(seg_ce/cnt) + lmbd * mean_k(seg_reg/(cnt*D))

v2 design (memory-bound target: 39.3 MB/core HBM reads ~= 117 us of wire time):
  - NO on-device collective: each core emits its raw [10, 67] f32 PSUM segment
    accumulator (cols 0:64 per-d erf seg-sums, 64 picked, 65 counts, 66 lse);
    the host sums the 8 cores and does the ~50-flop finish in numpy. This cuts
    the ~44 us tail (AllGather latency + serial scalar DVE chain) of v1.
  - DMA: u streams as 16 x 2 MB chunks on the sync HWDGE queue (contiguous
    16 KB per-partition packets); yg + yhat are two big t0 DMAs on the scalar
    HWDGE queue (40 KB / 4 KB packets) so the exp/ln excursion happens early.
  - ACT order is explicitly chained (the Tile scheduler would otherwise
    interleave and each function switch costs a ~1.5 us ACT table load):
    warm-erf, erf c0, erf c1, exp x4 slabs, ln x1, erf c2..c15  -> 4 table
    loads total (v1 had 20).
  - per 64-row chunk one bf16 work tile [128, 64, 67]:
      cols 0:64  erf(sqrt2*u + sqrt2/2)    (ACT, strided out; 0.5+0.5* affine
                 folded into the host fixup: seg_f = 0.5*D*cnt + 0.5*seg_erf)
      col  64    picked = sum_c onehot*yg  (DVE row-reduce done early per slab)
      col  65    ones                      (counts)
      col  66    lse                       (ACT ln early into a flat tile,
                                            bf16 col copy when the buffer frees)
  - PE: per 128-row group one ldweights(onehot[128,10]) + matmul(work[128,67]),
    accumulating PSUM [10,67] across all 1024 groups; onehot lives in one big
    bf16 tile computed early from yhat via iota-compare.
"""

import math
import sys

if "/opt/trn_rl_repo" not in sys.path:
    sys.path.insert(0, "/opt/trn_rl_repo")

import numpy as np

N_CORES = 8
N_FULL = 1048576
C = 10
D = 64
P = 128
ROWS_PER_CORE = N_FULL // N_CORES  # 131072
SQ2 = math.sqrt(2.0)
W_COLS = D + 3  # erf block | picked | ones | lse

COL_PICK = D
COL_ONES = D + 1
COL_LSE = D + 2


def build(rows_per_core=ROWS_PER_CORE, w=64, n_slab=4, u_bufs=4, w_bufs=4):
    from concourse import bacc, mybir, tile

    f32 = mybir.dt.float32
    bf16 = mybir.dt.bfloat16
    i32 = mybir.dt.int32
    FT = mybir.ActivationFunctionType
    ALU = mybir.AluOpType
    AX = mybir.AxisListType

    rpp = rows_per_core // P  # rows per partition (1024)
    assert rpp * P == rows_per_core
    nch = rpp // w  # chunks (16)
    assert nch * w == rpp
    slab = rpp // n_slab  # exp/onehot slab rows (256)
    assert slab * n_slab == rpp

    nc = bacc.Bacc("TRN2", target_bir_lowering=False, debug=False, num_devices=N_CORES)

    yh_d = nc.dram_tensor("yhat", [rows_per_core], i32, kind="ExternalInput")
    yg_d = nc.dram_tensor("yg", [rows_per_core, C], f32, kind="ExternalInput")
    u_d = nc.dram_tensor("u_zg", [rows_per_core, D], f32, kind="ExternalInput")
    out_d = nc.dram_tensor("out", [C, W_COLS], f32, kind="ExternalOutput")

    u_v = u_d[:].rearrange("(p r) d -> p r d", p=P)
    yg_v = yg_d[:].rearrange("(p r) c -> p r c", p=P)
    yh_v = yh_d[:].rearrange("(p r) -> p r", p=P)

    from concourse.tile_rust import add_dep_helper

    last_sc = [None]

    def sc_ordered(inst):
        """Chain scalar-engine instructions in program order so the Tile
        scheduler cannot interleave ACT functions (each interleave costs a
        ~1.5 us ACT table-set load) or delay the t0 HWDGE DMA issues."""
        raw = getattr(inst, "ins", inst)
        if last_sc[0] is not None:
            add_dep_helper(raw, last_sc[0], sync=True, reason="scalar order")
        last_sc[0] = raw
        return inst

    with tile.TileContext(nc) as tc:
        with (
            tc.tile_pool(name="const", bufs=1) as constp,
            tc.tile_pool(name="io", bufs=1) as iop,
            tc.tile_pool(name="work", bufs=1) as workp,
            tc.tile_pool(name="psum", bufs=1, space="PSUM") as psump,
        ):
            # --- t0 DMAs ---
            # u chunks on the sync HWDGE queue (16 KB/partition packets)
            u_ts = {}
            for ci in range(nch):
                u_t = iop.tile([P, w, D], f32, name="u_t", bufs=u_bufs)
                nc.sync.dma_start(u_t[:], u_v[:, ci * w : (ci + 1) * w, :])
                u_ts[ci] = u_t
            # yhat + yg as two big DMAs on the scalar HWDGE queue
            yh_i = constp.tile([P, rpp], i32)
            sc_ordered(nc.scalar.dma_start(yh_i[:], yh_v))
            yg_t = iop.tile([P, rpp, C], f32)
            sc_ordered(nc.scalar.dma_start(yg_t[:], yg_v))

            # --- constants ---
            erf_bias = constp.tile([P, 1], f32)
            nc.vector.memset(erf_bias[:], 0.5 * SQ2)
            iota_f = constp.tile([P, 1, C], f32)
            nc.gpsimd.iota(
                iota_f[:, 0, :], [[1, C]],
                channel_multiplier=0, allow_small_or_imprecise_dtypes=True,
            )
            yh_f = constp.tile([P, rpp], f32)
            nc.vector.tensor_copy(yh_f[:], yh_i[:])

            # warm the erf table while u chunk 0 is in flight
            warm_act = constp.tile([1, 1], f32)
            nc.vector.memset(warm_act[:], 1.0)
            wa_o = constp.tile([1, 1], f32)
            sc_ordered(
                nc.scalar.activation(
                    wa_o[:], warm_act[:], FT.Erf, bias=erf_bias[0:1, :], scale=SQ2
                )
            )

            # --- early DVE path (gated on yhat/yg only): onehot, picked, and
            # (after exp) sumexp; all in n_slab big slabs ---
            ohbig = constp.tile([P, rpp, C], bf16)
            picked = constp.tile([P, rpp], f32)
            sume = constp.tile([P, rpp], f32)
            lse16 = constp.tile([P, rpp], f32)
            pg_ts = {}
            for s in range(n_slab):
                s0, s1 = s * slab, (s + 1) * slab
                nc.vector.tensor_tensor(
                    ohbig[:, s0:s1, :],
                    yh_f[:, s0:s1].broadcast_to([P, slab, C]),
                    iota_f[:].broadcast_to([P, slab, C]),
                    ALU.is_equal,
                )
                pg_t = workp.tile([P, slab, C], bf16, name="pg_t", bufs=2)
                nc.vector.tensor_tensor(
                    pg_t[:], ohbig[:, s0:s1, :], yg_t[:, s0:s1, :], ALU.mult
                )
                nc.vector.reduce_sum(picked[:, s0:s1], pg_t[:], axis=AX.X)
                pg_ts[s] = pg_t

            work_ts = {}

            def do_erf(ci):
                work_t = workp.tile([P, w, W_COLS], bf16, name="work_t", bufs=w_bufs)
                sc_ordered(
                    nc.scalar.activation(
                        work_t[:, :, 0:D], u_ts.pop(ci)[:], FT.Erf,
                        bias=erf_bias[:], scale=SQ2,
                    )
                )
                work_ts[ci] = work_t

            # erf chunks 0..1 while yg lands
            do_erf(0)
            do_erf(1)

            # exp (in-place, after the slab's picked mult consumed raw yg),
            # then row-sumexp on DVE, then one big ln
            for s in range(n_slab):
                s0, s1 = s * slab, (s + 1) * slab
                sc_ordered(
                    nc.scalar.activation(yg_t[:, s0:s1, :], yg_t[:, s0:s1, :], FT.Exp)
                )
                nc.vector.reduce_sum(sume[:, s0:s1], yg_t[:, s0:s1, :], axis=AX.X)
            sc_ordered(nc.scalar.activation(lse16[:], sume[:], FT.Ln))

            # the rest of the erf stream
            for ci in range(2, nch):
                do_erf(ci)

            # --- per-chunk side columns + PE segment accumulation ---
            ps = psump.tile([C, W_COLS], f32)
            for ci in range(nch):
                r0, r1 = ci * w, (ci + 1) * w
                work_t = work_ts.pop(ci)
                nc.vector.tensor_copy(work_t[:, :, COL_PICK], picked[:, r0:r1])
                nc.vector.memset(work_t[:, :, COL_ONES], 1.0)
                nc.vector.tensor_copy(work_t[:, :, COL_LSE], lse16[:, r0:r1])
                for g in range(w):
                    first = ci == 0 and g == 0
                    last = ci == nch - 1 and g == w - 1
                    nc.tensor.matmul(
                        ps[:], ohbig[:, r0 + g, :], work_t[:, g, :],
                        start=first, stop=last,
                    )

            # --- emit the raw accumulator; host finishes ---
            accS = constp.tile([C, W_COLS], f32)
            nc.vector.tensor_copy(accS[:], ps[:])
            nc.sync.dma_start(out_d[:], accS[:])

    nc.compile()
    return nc


_NC_CACHE = {}


def _get_nc(**kw):
    key = tuple(sorted(kw.items()))
    if key not in _NC_CACHE:
        _NC_CACHE[key] = build(**kw)
    return _NC_CACHE[key]


def make_in_maps(yhat, yg, u_zg, rows_per_core=ROWS_PER_CORE):
    yhat = np.ascontiguousarray(np.asarray(yhat).astype(np.int32))
    yg = np.ascontiguousarray(np.asarray(yg, dtype=np.float32))
    u_zg = np.ascontiguousarray(np.asarray(u_zg, dtype=np.float32))
    n = yhat.shape[0]
    assert n == rows_per_core * N_CORES
    in_maps = []
    for i in range(N_CORES):
        s = slice(i * rows_per_core, (i + 1) * rows_per_core)
        in_maps.append({"yhat": yhat[s], "yg": yg[s], "u_zg": u_zg[s]})
    return in_maps


def _finish(acc_sum, lmbd):
    """acc_sum: [C, W_COLS] f64 summed over cores. ~50 flops in numpy."""
    seg_erf = acc_sum[:, 0:D].sum(axis=1)
    seg_pick = acc_sum[:, COL_PICK]
    cnt = acc_sum[:, COL_ONES]
    seg_lse = acc_sum[:, COL_LSE]
    present = cnt > 0
    denom = np.where(present, cnt, 1.0)
    seg_reg = 0.5 * D * cnt + 0.5 * seg_erf
    reg_c = seg_reg / (denom * D)
    aux_c = (seg_lse - seg_pick) / denom
    n_unique = present.sum()
    reg = np.where(present, reg_c, 0.0).sum() / n_unique
    aux = np.where(present, aux_c, 0.0).sum() / n_unique
    return np.float32(aux + float(lmbd) * reg)


def run(yhat, yg, u_zg, lmbd, trace=False, rows_per_core=ROWS_PER_CORE, **kw):
    from concourse import bass_utils

    nc = _get_nc(rows_per_core=rows_per_core, **kw)
    in_maps = make_in_maps(yhat, yg, u_zg, rows_per_core)
    res = bass_utils.run_bass_kernel_spmd(
        nc, in_maps, core_ids=list(range(N_CORES)), trace=trace
    )
    acc = np.zeros((C, W_COLS), dtype=np.float64)
    for r in res.results:
        acc += np.asarray(r["out"], dtype=np.float64)
    val = _finish(acc, lmbd)
    return val, res


def kernel(yhat, yg, u_zg, lmbd):
    val, _ = run(yhat, yg, u_zg, lmbd)
    return np.asarray(val, dtype=np.float32).reshape(())


# revision 5
# speedup vs baseline: 1.4048x; 1.1260x over previous
"""Trainium2 Bass kernel for nn_AuxLoss (aux CE loss + erf regularizer, segment-
mean over K=10 classes), data-parallel over 8 NeuronCores.

Math (per reference):
  f(u)      = 0.5 - 0.5*erf((-0.5 - u)/(sigma*sqrt2)) = 0.5 + 0.5*erf(sqrt2*u + sqrt2/2)
  row_reg_n = sum_d f(u[n,d])
  row_ce_n  = logsumexp(yg[n,:]) - yg[n, yhat[n]]
  per-class means over rows with yhat==k, averaged over present classes:
  out = mean_k(seg_ce/cnt) + lmbd * mean_k(seg_reg/(cnt*D))

v4 design. Measured facts driving it: one HWDGE queue sustains ~350 GB/s and
the two together ~575 GB/s, so with u (33.6 MB) striped across both queues the
wire is no longer the limit -- the ACT engine's erf stream (16 x 3.7 us) is.
Everything else is arranged to never stall ACT:
  - NO on-device collective: each core emits its raw [10, 76] f32 segment
    accumulator; the host sums the 8 cores and does the ~50-flop finish in
    numpy (the legitimate unshard step). v1's AllGather cost a ~44 us tail.
  - DMA issues are NOT chained to the ACT stream (a dep on a DMA instruction
    waits for the *transfer*, which in v2/v3 pushed the first erf to ~50 us).
    sync queue: yhat, yg x4 slabs, u chunks 8..15; scalar queue: u chunks
    0..7 (first four issued at t0 into fresh buffers).
  - ACT order explicitly chained: warm-erf, erf c0, erf c1, exp x8 halves,
    ln, erf c2..c15 (last chunk in 2 halves to shorten the PE tail)
    -> 4 table loads total.
  - exp reads raw yg but writes a separate rotating [128,128,10] tile (so it
    is gated ONLY by the yg DMA, not by any consumer of raw yg); DVE row-sums
    it; one ln produces lse for all rows.
  - per 64-row chunk one bf16 work tile [128, 64, 75]:
      cols 0:64  erf(sqrt2*u + sqrt2/2)   (ACT, strided out; the 0.5+0.5*
                 affine is folded into the host fixup)
      cols 64:74 onehot*yg                (DVE; row-sum of the matmul's
                 [10,10] block = diagonal = picked, done on host)
      col  74    ones                     (counts)
  - PE: per 128-row group ldweights(onehot[128,10]) + matmul(work[128,75]),
    accumulating PSUM [10,75] over all 1024 groups; onehot lives in one big
    bf16 tile computed early from yhat via iota-compare.
  - lse per-class segment sum stays off the PE/work-buffer stream (DVE
    class-major masked reduce into [128,10], folded by one ones-moving
    matmul into PSUM [10,1] mid-stream).
"""

import math
import sys

if "/opt/trn_rl_repo" not in sys.path:
    sys.path.insert(0, "/opt/trn_rl_repo")

import numpy as np

N_CORES = 8
N_FULL = 1048576
C = 10
D = 64
P = 128
ROWS_PER_CORE = N_FULL // N_CORES  # 131072
SQ2 = math.sqrt(2.0)
W_COLS = D + C + 1  # erf block | onehot*yg block | ones
W_OUT = W_COLS + 1  # + lse col appended in the output tile

COL_ONES = D + C


def build(rows_per_core=ROWS_PER_CORE, w=64, n_slab=4, n_half=8, u_bufs=4,
          w_bufs=4, n_scalar_u=8):
    from concourse import bacc, mybir, tile

    f32 = mybir.dt.float32
    bf16 = mybir.dt.bfloat16
    i32 = mybir.dt.int32
    FT = mybir.ActivationFunctionType
    ALU = mybir.AluOpType
    AX = mybir.AxisListType

    rpp = rows_per_core // P  # rows per partition (1024)
    assert rpp * P == rows_per_core
    nch = rpp // w  # chunks (16)
    assert nch * w == rpp
    slab = rpp // n_slab  # onehot/lse slab rows (256)
    assert slab * n_slab == rpp
    half = rpp // n_half  # exp slab rows (128)
    assert half * n_half == rpp

    nc = bacc.Bacc("TRN2", target_bir_lowering=False, debug=False, num_devices=N_CORES)

    yh_d = nc.dram_tensor("yhat", [rows_per_core], i32, kind="ExternalInput")
    yg_d = nc.dram_tensor("yg", [rows_per_core, C], f32, kind="ExternalInput")
    u_d = nc.dram_tensor("u_zg", [rows_per_core, D], f32, kind="ExternalInput")
    out_d = nc.dram_tensor("out", [C, W_OUT], f32, kind="ExternalOutput")

    u_v = u_d[:].rearrange("(p r) d -> p r d", p=P)
    yg_v = yg_d[:].rearrange("(p r) c -> p r c", p=P)
    yh_v = yh_d[:].rearrange("(p r) -> p r", p=P)

    from concourse.tile_rust import add_dep_helper

    last_sc = [None]

    def sc_ordered(inst):
        """Chain ACT instructions so the Tile scheduler cannot interleave ACT
        functions (each interleave costs a ~1.3 us ACT table-set load)."""
        raw = getattr(inst, "ins", inst)
        if last_sc[0] is not None:
            add_dep_helper(raw, last_sc[0], sync=True, reason="act order")
        last_sc[0] = raw
        return inst

    with tile.TileContext(nc) as tc:
        with (
            tc.tile_pool(name="const", bufs=1) as constp,
            tc.tile_pool(name="io", bufs=1) as iop,
            tc.tile_pool(name="work", bufs=1) as workp,
            tc.tile_pool(name="psum", bufs=1, space="PSUM") as psump,
        ):
            # --- t0 DMA issues (unchained) ---
            # scalar HWDGE queue: u chunks 0..n_scalar_u-1
            u_ts = {}
            for ci in range(n_scalar_u):
                u_t = iop.tile([P, w, D], f32, name="u_t", bufs=u_bufs)
                nc.scalar.dma_start(u_t[:], u_v[:, ci * w : (ci + 1) * w, :])
                u_ts[ci] = u_t
            # sync HWDGE queue: yhat, yg slabs, u chunks n_scalar_u..15
            yh_i = constp.tile([P, rpp], i32)
            nc.sync.dma_start(yh_i[:], yh_v)
            yg_t = iop.tile([P, rpp, C], f32)
            for s in range(n_slab):
                s0, s1 = s * slab, (s + 1) * slab
                nc.sync.dma_start(yg_t[:, s0:s1, :], yg_v[:, s0:s1, :])
            for ci in range(n_scalar_u, nch):
                u_t = iop.tile([P, w, D], f32, name="u_t", bufs=u_bufs)
                nc.sync.dma_start(u_t[:], u_v[:, ci * w : (ci + 1) * w, :])
                u_ts[ci] = u_t

            # --- constants ---
            erf_bias = constp.tile([P, 1], f32)
            nc.vector.memset(erf_bias[:], 0.5 * SQ2)
            ones128 = constp.tile([P, 1], f32)
            nc.vector.memset(ones128[:], 1.0)
            iota_f = constp.tile([P, 1, C], f32)
            nc.gpsimd.iota(
                iota_f[:, 0, :], [[1, C]],
                channel_multiplier=0, allow_small_or_imprecise_dtypes=True,
            )
            yh_f = constp.tile([P, rpp], f32)
            nc.vector.tensor_copy(yh_f[:], yh_i[:])

            # warm the erf table while u chunk 0 is in flight
            warm_act = constp.tile([1, 1], f32)
            nc.vector.memset(warm_act[:], 1.0)
            wa_o = constp.tile([1, 1], f32)
            sc_ordered(
                nc.scalar.activation(
                    wa_o[:], warm_act[:], FT.Erf, bias=erf_bias[0:1, :], scale=SQ2
                )
            )

            # onehot, slab-wise (gated on yhat only)
            ohbig = constp.tile([P, rpp, C], bf16)
            for s in range(n_slab):
                s0, s1 = s * slab, (s + 1) * slab
                nc.vector.tensor_tensor(
                    ohbig[:, s0:s1, :],
                    yh_f[:, s0:s1].broadcast_to([P, slab, C]),
                    iota_f[:].broadcast_to([P, slab, C]),
                    ALU.is_equal,
                )

            work_ts = {}

            def do_erf(ci, parts=1):
                work_t = workp.tile([P, w, W_COLS], bf16, name="work_t", bufs=w_bufs)
                u_t = u_ts.pop(ci)
                step = w // parts
                for k in range(parts):
                    r0, r1 = k * step, (k + 1) * step
                    sc_ordered(
                        nc.scalar.activation(
                            work_t[:, r0:r1, 0:D], u_t[:, r0:r1, :], FT.Erf,
                            bias=erf_bias[:], scale=SQ2,
                        )
                    )
                work_ts[ci] = work_t

            # erf chunks 0..1 while yg lands
            do_erf(0)
            do_erf(1)

            # exp (raw yg -> separate rotating tile, gated only on the yg DMA),
            # DVE row-sumexp, one ln for all rows
            sume = constp.tile([P, rpp], f32)
            lse16 = constp.tile([P, rpp], f32)
            for h in range(n_half):
                h0, h1 = h * half, (h + 1) * half
                yge = workp.tile([P, half, C], f32, name="yge", bufs=2)
                sc_ordered(
                    nc.scalar.activation(yge[:], yg_t[:, h0:h1, :], FT.Exp)
                )
                nc.vector.reduce_sum(sume[:, h0:h1], yge[:], axis=AX.X)
            sc_ordered(nc.scalar.activation(lse16[:], sume[:], FT.Ln))

            # the rest of the erf stream (last chunk in halves for the tail)
            for ci in range(2, nch):
                do_erf(ci, parts=2 if ci == nch - 1 else 1)

            # --- lse per-class segment sum, slab-wise, off the PE stream ---
            # lacc[p, k] += sum_r onehot[p, r, k] * lse[p, r]
            lacc = constp.tile([P, C], f32)
            nc.vector.memset(lacc[:], 0.0)
            for s in range(n_slab):
                s0, s1 = s * slab, (s + 1) * slab
                ltmp = workp.tile([P, C, slab], bf16, name="ltmp", bufs=1)
                nc.vector.tensor_tensor(
                    ltmp[:],
                    ohbig[:, s0:s1, :].transpose([0, 2, 1]),
                    lse16[:, s0:s1].broadcast_to([P, slab, C]).transpose([0, 2, 1]),
                    ALU.mult,
                )
                lred = workp.tile([P, C], f32, name="lred", bufs=1)
                nc.vector.reduce_sum(lred[:], ltmp[:], axis=AX.X)
                nc.vector.tensor_tensor(lacc[:], lacc[:], lred[:], ALU.add)

            # --- per-chunk side columns + PE segment accumulation ---
            ps = psump.tile([C, W_COLS], f32)
            ps_l = psump.tile([C, 1], f32)
            for ci in range(nch):
                r0, r1 = ci * w, (ci + 1) * w
                work_t = work_ts.pop(ci)
                nc.vector.tensor_tensor(
                    work_t[:, :, D : D + C], ohbig[:, r0:r1, :],
                    yg_t[:, r0:r1, :], ALU.mult,
                )
                nc.vector.memset(work_t[:, :, COL_ONES], 1.0)
                for g in range(w):
                    first = ci == 0 and g == 0
                    last = ci == nch - 1 and g == w - 1
                    nc.tensor.matmul(
                        ps[:], ohbig[:, r0 + g, :], work_t[:, g, :],
                        start=first, stop=last,
                    )

            # partition-reduce the lse accumulator: lacc.T @ ones -> [10, 1]
            nc.tensor.matmul(ps_l[:], lacc[:], ones128[:], start=True, stop=True)

            # --- emit the raw accumulator; host finishes ---
            accS = constp.tile([C, W_OUT], f32)
            nc.vector.tensor_copy(accS[:, 0:W_COLS], ps[:])
            nc.vector.tensor_copy(accS[:, W_COLS : W_OUT], ps_l[:])
            nc.sync.dma_start(out_d[:], accS[:])

    nc.compile()
    return nc


_NC_CACHE = {}


def _get_nc(**kw):
    key = tuple(sorted(kw.items()))
    if key not in _NC_CACHE:
        _NC_CACHE[key] = build(**kw)
    return _NC_CACHE[key]


def make_in_maps(yhat, yg, u_zg, rows_per_core=ROWS_PER_CORE):
    yhat = np.ascontiguousarray(np.asarray(yhat).astype(np.int32))
    yg = np.ascontiguousarray(np.asarray(yg, dtype=np.float32))
    u_zg = np.ascontiguousarray(np.asarray(u_zg, dtype=np.float32))
    n = yhat.shape[0]
    assert n == rows_per_core * N_CORES
    in_maps = []
    for i in range(N_CORES):
        s = slice(i * rows_per_core, (i + 1) * rows_per_core)
        in_maps.append({"yhat": yhat[s], "yg": yg[s], "u_zg": u_zg[s]})
    return in_maps


def _finish(acc_sum, lmbd):
    """acc_sum: [C, W_OUT] f64 summed over cores. ~100 flops in numpy."""
    seg_erf = acc_sum[:, 0:D].sum(axis=1)
    seg_pick = acc_sum[:, D : D + C].sum(axis=1)  # row-sum of the block = diag
    cnt = acc_sum[:, COL_ONES]
    seg_lse = acc_sum[:, W_COLS]
    present = cnt > 0
    denom = np.where(present, cnt, 1.0)
    seg_reg = 0.5 * D * cnt + 0.5 * seg_erf
    reg_c = seg_reg / (denom * D)
    aux_c = (seg_lse - seg_pick) / denom
    n_unique = present.sum()
    reg = np.where(present, reg_c, 0.0).sum() / n_unique
    aux = np.where(present, aux_c, 0.0).sum() / n_unique
    return np.float32(aux + float(lmbd) * reg)


def run(yhat, yg, u_zg, lmbd, trace=False, rows_per_core=ROWS_PER_CORE, **kw):
    from concourse import bass_utils

    nc = _get_nc(rows_per_core=rows_per_core, **kw)
    in_maps = make_in_maps(yhat, yg, u_zg, rows_per_core)
    res = bass_utils.run_bass_kernel_spmd(
        nc, in_maps, core_ids=list(range(N_CORES)), trace=trace
    )
    acc = np.zeros((C, W_OUT), dtype=np.float64)
    for r in res.results:
        acc += np.asarray(r["out"], dtype=np.float64)
    val = _finish(acc, lmbd)
    return val, res


def kernel(yhat, yg, u_zg, lmbd):
    val, _ = run(yhat, yg, u_zg, lmbd)
    return np.asarray(val, dtype=np.float32).reshape(())


# revision 27
# speedup vs baseline: 1.4332x; 1.0202x over previous
"""Trainium2 Bass kernel for nn_AuxLoss (aux CE loss + erf regularizer, segment-
mean over K=10 classes), data-parallel over 8 NeuronCores.

Math (per reference):
  f(u)      = 0.5 - 0.5*erf((-0.5 - u)/(sigma*sqrt2)) = 0.5 + 0.5*erf(sqrt2*u + sqrt2/2)
  row_reg_n = sum_d f(u[n,d])
  row_ce_n  = logsumexp(yg[n,:]) - yg[n, yhat[n]]
  per-class means over rows with yhat==k, averaged over present classes:
  out = aux + lmbd * reg

v10 design. Measured facts: each HWDGE queue sustains ~270-350 GB/s and the
pair ~530+ GB/s, so the wire is not the limit -- the single ACT engine is
(erf 16 x 3.71 us at work-stride ~66, exp 8 x 1.63 us, ln 1.2 us). The
kernel is built around an uninterrupted ACT stream:
  - ACT chain: warm-exp, exp x8 (one per 128-row yg piece, gated only on its
    own DMA), ln, then erf c0..c15 back-to-back (first and last chunks in 2
    pieces: the first so erf starts as soon as 1 MB has landed, the last to
    shorten the PE tail). exp and ln share one ACT table set, so the whole
    kernel pays exactly 3 table loads (exp prewarmed at t~0, ln, erf).
    Because ln completes before the first erf, lse can ride the work tile
    like every other per-row quantity -- no separate segment path at all.
  - Scheduler-proofing (v5-v8 deadlocked or mis-scheduled): u DMA issues for
    chunks >= u_bufs carry a branch dependency on the erf whose completion
    frees their pool slot, so the counting cap-gate is satisfied by
    construction and slots are requested in chunk order; every DVE
    instruction sits on one explicit chain in dependency-forward order. No
    dependency ever lands ON a DMA instruction (that waits for the
    *transfer*, which cost v2/v3 ~35 us of ACT delay).
  - DMA: scalar HWDGE queue carries u0 (2 pieces), u1, u2, then odd chunks;
    sync HWDGE queue carries the 8 yg pieces first (full-resident pool, no
    WARs) then even u chunks. Tail chunks alternate queues so per-queue
    transfer pace (5.9 us) beats the 2-chunks-per-queue erf pace (7.4 us).
    yhat rides the gpsimd SWDGE queue, casting i32->f32 in flight.
  - NO on-device collective: each core emits its raw [10, 67] f32 segment
    accumulator; the host sums the 8 cores and does the ~50-flop finish in
    numpy (the legitimate unshard step). v1's AllGather cost a ~44 us tail.
  - per 64-row chunk one bf16 work tile [128, 64, 67]:
      cols 0:64  erf(sqrt2*u + sqrt2/2)    (ACT, strided out; the 0.5+0.5*
                 affine is folded into the host fixup)
      col  64    picked = sum_c onehot*yg  (DVE row-reduce of onehot*raw yg
                                            straight into the work column)
      col  65    ones                      (counts)
      col  66    lse                       (DVE bf16 copy from the ln output)
  - PE: per 128-row group ldweights(onehot[128,10]) + matmul(work[128,67]),
    accumulating PSUM [10,67] over all 1024 groups; onehot lives in one big
    bf16 tile computed early from yhat via iota-compare.
"""

import math
import sys

if "/opt/trn_rl_repo" not in sys.path:
    sys.path.insert(0, "/opt/trn_rl_repo")

import numpy as np

N_CORES = 8
N_FULL = 1048576
C = 10
D = 64
P = 128
ROWS_PER_CORE = N_FULL // N_CORES  # 131072
SQ2 = math.sqrt(2.0)
W_COLS = D + 3  # erf block | picked | ones | lse
W_OUT = W_COLS

COL_PICK = D
COL_ONES = D + 1
COL_LSE = D + 2


def build(rows_per_core=ROWS_PER_CORE, w=64, n_slab=4, n_half=8, u_bufs=4,
          w_bufs=4, ltmp_on_gpsimd=False):
    from concourse import bacc, mybir, tile

    f32 = mybir.dt.float32
    bf16 = mybir.dt.bfloat16
    i32 = mybir.dt.int32
    FT = mybir.ActivationFunctionType
    ALU = mybir.AluOpType
    AX = mybir.AxisListType

    rpp = rows_per_core // P  # rows per partition (1024)
    assert rpp * P == rows_per_core
    nch = rpp // w  # chunks (16)
    assert nch * w == rpp
    slab = rpp // n_slab  # lse slab rows (256)
    assert slab * n_slab == rpp
    half = rpp // n_half  # yg piece rows (128)
    assert half * n_half == rpp
    assert w <= half and half % w == 0

    nc = bacc.Bacc("TRN2", target_bir_lowering=False, debug=False, num_devices=N_CORES)

    yh_d = nc.dram_tensor("yhat", [rows_per_core], i32, kind="ExternalInput")
    yg_d = nc.dram_tensor("yg", [rows_per_core, C], f32, kind="ExternalInput")
    u_d = nc.dram_tensor("u_zg", [rows_per_core, D], f32, kind="ExternalInput")
    out_d = nc.dram_tensor("out", [C, W_OUT], f32, kind="ExternalOutput")

    u_v = u_d[:].rearrange("(p r) d -> p r d", p=P)
    yg_v = yg_d[:].rearrange("(p r) c -> p r c", p=P)
    yh_v = yh_d[:].rearrange("(p r) -> p r", p=P)

    from concourse.tile_rust import add_dep_helper

    def mk_chain(box, reason):
        def link(inst):
            raw = getattr(inst, "ins", inst)
            if box[0] is not None:
                add_dep_helper(raw, box[0], sync=True, reason=reason)
            box[0] = raw
            return inst
        return link

    sc_box = [None]
    sc_ordered = mk_chain(sc_box, "act order")
    dve_ordered = mk_chain([None], "dve order")
    gp_ordered = mk_chain([None], "gpsimd order")

    with tile.TileContext(nc) as tc:
        with (
            tc.tile_pool(name="const", bufs=1) as constp,
            tc.tile_pool(name="io", bufs=1) as iop,
            tc.tile_pool(name="work", bufs=1) as workp,
            tc.tile_pool(name="psum", bufs=1, space="PSUM") as psump,
        ):
            # --- u tiles created in ci order. Chunks 0..3 use fresh pool
            # slots and are issued at t0; every chunk k >= u_bufs is issued
            # with a branch dependency on erf(k - u_bufs) -- the compute
            # instruction whose completion frees its slot -- so the counting
            # cap-gate is always already satisfied when an issue runs and
            # slots are requested in ci order on every engine (v6/v7
            # deadlocked when the scheduler let a late chunk's issue grab a
            # slot ahead of an early one, or parked an issue with an
            # unsatisfied cap-wait inside the ACT stream). No dependency
            # ever lands ON a DMA instruction (that would wait for the
            # transfer, which cost v2/v3 ~35 us of ACT delay). ---
            u_ts = {ci: iop.tile([P, w, D], f32, name="u_t", bufs=u_bufs)
                    for ci in range(nch)}
            h2 = w // 2

            def is_scalar_chunk(ci):
                return ci <= 2 or ci % 2 == 1

            def issue_u(ci, piece=None, after=None):
                eng = nc.scalar if is_scalar_chunk(ci) else nc.sync
                if piece is None:
                    r0, r1 = 0, w
                else:
                    r0, r1 = piece * h2, (piece + 1) * h2
                inst = eng.dma_start(
                    u_ts[ci][:, r0:r1, :],
                    u_v[:, ci * w + r0 : ci * w + r1, :],
                )
                if after is not None:
                    add_dep_helper(
                        getattr(inst, "ins", inst), after, sync=True,
                        reason="u slot freed by this erf",
                    )
                return inst

            # t0: chunks 0..2 on scalar (they feed the head of the erf
            # stream), yg on sync, chunk 3 on scalar (fresh slot)
            issue_u(0, piece=0)
            issue_u(0, piece=1)
            issue_u(1)
            issue_u(2)
            yg_ts = {}
            for h in range(n_half):
                yg_t = iop.tile([P, half, C], f32, name="yg_t", bufs=n_half)
                nc.sync.dma_start(yg_t[:], yg_v[:, h * half : (h + 1) * half, :])
                yg_ts[h] = yg_t
            issue_u(3)
            # yhat via gpsimd SWDGE, casting i32 -> f32 in flight
            yh_f = constp.tile([P, rpp], f32)
            gp_ordered(nc.gpsimd.dma_start(yh_f[:], yh_v))

            # --- constants ---
            erf_bias = constp.tile([P, 1], f32)
            nc.vector.memset(erf_bias[:], 0.5 * SQ2)
            iota_f = constp.tile([P, 1, C], f32)
            gp_ordered(nc.gpsimd.iota(
                iota_f[:, 0, :], [[1, C]],
                channel_multiplier=0, allow_small_or_imprecise_dtypes=True,
            ))

            # warm the exp table while the first DMAs are in flight
            warm_act = constp.tile([1, 1], f32)
            nc.vector.memset(warm_act[:], 1.0)
            wa_o = constp.tile([1, 1], f32)
            sc_ordered(nc.scalar.activation(wa_o[:], warm_act[:], FT.Exp))

            # --- ACT phase 1: exp per yg piece, then one ln (shared table).
            # The DVE sumexp reduces MUST be emitted before the ln: the Tile
            # dep tracker orders accesses by emission, so a read emitted
            # before its writers gets no dependency (this was the v9/v10
            # NaN). The onehot for slab 0 is emitted first so it heads the
            # DVE chain. ---
            sume = constp.tile([P, rpp], f32)
            lse16 = constp.tile([P, rpp], f32)
            ohbig = constp.tile([P, rpp, C], bf16)

            def do_oneh(s):
                s0, s1 = s * slab, (s + 1) * slab
                dve_ordered(nc.vector.tensor_tensor(
                    ohbig[:, s0:s1, :],
                    yh_f[:, s0:s1].broadcast_to([P, slab, C]),
                    iota_f[:].broadcast_to([P, slab, C]),
                    ALU.is_equal,
                ))

            do_oneh(0)
            yge_ts = {}
            for h in range(n_half):
                yge = workp.tile([P, half, C], bf16, name="yge", bufs=n_half)
                sc_ordered(nc.scalar.activation(yge[:], yg_ts[h][:], FT.Exp))
                yge_ts[h] = yge
                h0 = h * half
                dve_ordered(nc.vector.reduce_sum(
                    sume[:, h0 : h0 + half], yge[:], axis=AX.X
                ))
            sc_ordered(nc.scalar.activation(lse16[:], sume[:], FT.Ln))

            # --- ACT phase 2: the erf stream with chained scalar u issues ---
            work_ts = {}

            def do_erf(ci, parts=1):
                work_t = workp.tile([P, w, W_COLS], bf16, name="work_t", bufs=w_bufs)
                u_t = u_ts.pop(ci)
                step = w // parts
                for k in range(parts):
                    r0, r1 = k * step, (k + 1) * step
                    sc_ordered(
                        nc.scalar.activation(
                            work_t[:, r0:r1, 0:D], u_t[:, r0:r1, :], FT.Erf,
                            bias=erf_bias[:], scale=SQ2,
                        )
                    )
                work_ts[ci] = work_t

            for ci in range(nch):
                do_erf(ci, parts=2 if ci in (0, nch - 1) else 1)
                nxt = ci + u_bufs
                if nxt < nch:
                    issue_u(nxt, after=sc_box[0])

            # --- DVE chain continues: side cols c0..c3, oneh s1..s3, side
            # cols c4..c15, accS ---
            def do_side(ci):
                r0 = ci * w
                h = ci // (half // w)
                hr0 = (ci % (half // w)) * w
                pg_t = workp.tile([P, w, C], bf16, name="pg_t", bufs=2)
                dve_ordered(nc.vector.tensor_tensor(
                    pg_t[:], ohbig[:, r0 : r0 + w, :],
                    yg_ts[h][:, hr0 : hr0 + w, :], ALU.mult,
                ))
                with nc.allow_low_precision(reason="picked row has 1 nonzero"):
                    dve_ordered(nc.vector.reduce_sum(
                        work_ts[ci][:, :, COL_PICK], pg_t[:], axis=AX.X
                    ))
                dve_ordered(nc.vector.memset(work_ts[ci][:, :, COL_ONES], 1.0))
                r0 = ci * w
                dve_ordered(nc.vector.tensor_copy(
                    work_ts[ci][:, :, COL_LSE], lse16[:, r0 : r0 + w]
                ))

            chunks_per_slab = slab // w
            for ci in range(chunks_per_slab):
                do_side(ci)
            for s in range(1, n_slab):
                do_oneh(s)
            for ci in range(chunks_per_slab, nch):
                do_side(ci)

            # --- PE segment accumulation ---
            ps = psump.tile([C, W_COLS], f32)
            for ci in range(nch):
                r0 = ci * w
                work_t = work_ts.pop(ci)
                for g in range(w):
                    first = ci == 0 and g == 0
                    last = ci == nch - 1 and g == w - 1
                    nc.tensor.matmul(
                        ps[:], ohbig[:, r0 + g, :], work_t[:, g, :],
                        start=first, stop=last,
                    )

            # --- emit the raw accumulator; host finishes ---
            accS = constp.tile([C, W_OUT], f32)
            dve_ordered(nc.vector.tensor_copy(accS[:], ps[:]))
            nc.sync.dma_start(out_d[:], accS[:])

    nc.compile()
    return nc


_NC_CACHE = {}


def _get_nc(**kw):
    key = tuple(sorted(kw.items()))
    if key not in _NC_CACHE:
        _NC_CACHE[key] = build(**kw)
    return _NC_CACHE[key]


def make_in_maps(yhat, yg, u_zg, rows_per_core=ROWS_PER_CORE):
    yhat = np.ascontiguousarray(np.asarray(yhat).astype(np.int32))
    yg = np.ascontiguousarray(np.asarray(yg, dtype=np.float32))
    u_zg = np.ascontiguousarray(np.asarray(u_zg, dtype=np.float32))
    n = yhat.shape[0]
    assert n == rows_per_core * N_CORES
    in_maps = []
    for i in range(N_CORES):
        s = slice(i * rows_per_core, (i + 1) * rows_per_core)
        in_maps.append({"yhat": yhat[s], "yg": yg[s], "u_zg": u_zg[s]})
    return in_maps


def _finish(acc_sum, lmbd):
    """acc_sum: [C, W_OUT] f64 summed over cores. ~50 flops in numpy."""
    seg_erf = acc_sum[:, 0:D].sum(axis=1)
    seg_pick = acc_sum[:, COL_PICK]
    cnt = acc_sum[:, COL_ONES]
    seg_lse = acc_sum[:, COL_LSE]
    present = cnt > 0
    denom = np.where(present, cnt, 1.0)
    seg_reg = 0.5 * D * cnt + 0.5 * seg_erf
    reg_c = seg_reg / (denom * D)
    aux_c = (seg_lse - seg_pick) / denom
    n_unique = present.sum()
    reg = np.where(present, reg_c, 0.0).sum() / n_unique
    aux = np.where(present, aux_c, 0.0).sum() / n_unique
    return np.float32(aux + float(lmbd) * reg)


def run(yhat, yg, u_zg, lmbd, trace=False, rows_per_core=ROWS_PER_CORE, **kw):
    from concourse import bass_utils

    nc = _get_nc(rows_per_core=rows_per_core, **kw)
    in_maps = make_in_maps(yhat, yg, u_zg, rows_per_core)
    res = bass_utils.run_bass_kernel_spmd(
        nc, in_maps, core_ids=list(range(N_CORES)), trace=trace
    )
    acc = np.zeros((C, W_OUT), dtype=np.float64)
    for r in res.results:
        acc += np.asarray(r["out"], dtype=np.float64)
    val = _finish(acc, lmbd)
    return val, res


def kernel(yhat, yg, u_zg, lmbd):
    val, _ = run(yhat, yg, u_zg, lmbd)
    return np.asarray(val, dtype=np.float32).reshape(())


# revision 29
# speedup vs baseline: 1.5257x; 1.0645x over previous
"""Trainium2 Bass kernel for nn_AuxLoss (aux CE loss + erf regularizer, segment-
mean over K=10 classes), data-parallel over 8 NeuronCores.

Math (per reference):
  f(u)      = 0.5 - 0.5*erf((-0.5 - u)/(sigma*sqrt2)) = 0.5 + 0.5*erf(sqrt2*u + sqrt2/2)
  row_reg_n = sum_d f(u[n,d])
  row_ce_n  = logsumexp(yg[n,:]) - yg[n, yhat[n]]
  per-class means over rows with yhat==k, averaged over present classes:
  out = aux + lmbd * reg

v10 design. Measured facts: each HWDGE queue sustains ~270-350 GB/s and the
pair ~530+ GB/s, so the wire is not the limit -- the single ACT engine is
(erf 16 x 3.71 us at work-stride ~66, exp 8 x 1.63 us, ln 1.2 us). The
kernel is built around an uninterrupted ACT stream:
  - ACT chain: warm-exp, exp x8 (one per 128-row yg piece, gated only on its
    own DMA), ln, then erf c0..c15 back-to-back (first and last chunks in 2
    pieces: the first so erf starts as soon as 1 MB has landed, the last to
    shorten the PE tail). exp and ln share one ACT table set, so the whole
    kernel pays exactly 3 table loads (exp prewarmed at t~0, ln, erf).
    Because ln completes before the first erf, lse can ride the work tile
    like every other per-row quantity -- no separate segment path at all.
  - Scheduler-proofing (v5-v8 deadlocked or mis-scheduled): u DMA issues for
    chunks >= u_bufs carry a branch dependency on the erf whose completion
    frees their pool slot, so the counting cap-gate is satisfied by
    construction and slots are requested in chunk order; every DVE
    instruction sits on one explicit chain in dependency-forward order. No
    dependency ever lands ON a DMA instruction (that waits for the
    *transfer*, which cost v2/v3 ~35 us of ACT delay).
  - DMA: scalar HWDGE queue carries u0 (2 pieces), u1, u2, then odd chunks;
    sync HWDGE queue carries the 8 yg pieces first (full-resident pool, no
    WARs) then even u chunks. Tail chunks alternate queues so per-queue
    transfer pace (5.9 us) beats the 2-chunks-per-queue erf pace (7.4 us).
    yhat rides the gpsimd SWDGE queue, casting i32->f32 in flight.
  - NO on-device collective: each core emits its raw [10, 67] f32 segment
    accumulator; the host sums the 8 cores and does the ~50-flop finish in
    numpy (the legitimate unshard step). v1's AllGather cost a ~44 us tail.
  - per 64-row chunk one bf16 work tile [128, 64, 67]:
      cols 0:64  erf(sqrt2*u + sqrt2/2)    (ACT, strided out; the 0.5+0.5*
                 affine is folded into the host fixup)
      col  64    picked = sum_c onehot*yg  (DVE row-reduce of onehot*raw yg
                                            straight into the work column)
      col  65    ones                      (counts)
      col  66    lse                       (DVE bf16 copy from the ln output)
  - PE: per 128-row group ldweights(onehot[128,10]) + matmul(work[128,67]),
    accumulating PSUM [10,67] over all 1024 groups; onehot lives in one big
    bf16 tile computed early from yhat via iota-compare.
"""

import math
import sys

if "/opt/trn_rl_repo" not in sys.path:
    sys.path.insert(0, "/opt/trn_rl_repo")

import numpy as np

N_CORES = 8
N_FULL = 1048576
C = 10
D = 64
P = 128
ROWS_PER_CORE = N_FULL // N_CORES  # 131072
SQ2 = math.sqrt(2.0)
W_COLS = D + 3  # erf block | picked | ones | lse
W_OUT = W_COLS

COL_PICK = D
COL_ONES = D + 1
COL_LSE = D + 2


def build(rows_per_core=ROWS_PER_CORE, w=64, n_slab=4, n_half=8, u_bufs=5,
          w_bufs=4, yge_bufs=4):
    from concourse import bacc, mybir, tile

    f32 = mybir.dt.float32
    bf16 = mybir.dt.bfloat16
    i32 = mybir.dt.int32
    FT = mybir.ActivationFunctionType
    ALU = mybir.AluOpType
    AX = mybir.AxisListType

    rpp = rows_per_core // P  # rows per partition (1024)
    assert rpp * P == rows_per_core
    nch = rpp // w  # chunks (16)
    assert nch * w == rpp
    slab = rpp // n_slab  # lse slab rows (256)
    assert slab * n_slab == rpp
    half = rpp // n_half  # yg piece rows (128)
    assert half * n_half == rpp
    assert w <= half and half % w == 0

    nc = bacc.Bacc("TRN2", target_bir_lowering=False, debug=False, num_devices=N_CORES)

    yh_d = nc.dram_tensor("yhat", [rows_per_core], i32, kind="ExternalInput")
    yg_d = nc.dram_tensor("yg", [rows_per_core, C], f32, kind="ExternalInput")
    u_d = nc.dram_tensor("u_zg", [rows_per_core, D], f32, kind="ExternalInput")
    out_d = nc.dram_tensor("out", [C, W_OUT], f32, kind="ExternalOutput")

    u_v = u_d[:].rearrange("(p r) d -> p r d", p=P)
    yg_v = yg_d[:].rearrange("(p r) c -> p r c", p=P)
    yh_v = yh_d[:].rearrange("(p r) -> p r", p=P)

    from concourse.tile_rust import add_dep_helper

    def mk_chain(box, reason):
        def link(inst):
            raw = getattr(inst, "ins", inst)
            if box[0] is not None:
                add_dep_helper(raw, box[0], sync=True, reason=reason)
            box[0] = raw
            return inst
        return link

    sc_box = [None]
    sc_ordered = mk_chain(sc_box, "act order")
    dve_ordered = mk_chain([None], "dve order")
    gp_ordered = mk_chain([None], "gpsimd order")

    with tile.TileContext(nc) as tc:
        with (
            tc.tile_pool(name="const", bufs=1) as constp,
            tc.tile_pool(name="io", bufs=1) as iop,
            tc.tile_pool(name="work", bufs=1) as workp,
            tc.tile_pool(name="psum", bufs=1, space="PSUM") as psump,
        ):
            # --- u tiles created in ci order. Chunks 0..3 use fresh pool
            # slots and are issued at t0; every chunk k >= u_bufs is issued
            # with a branch dependency on erf(k - u_bufs) -- the compute
            # instruction whose completion frees its slot -- so the counting
            # cap-gate is always already satisfied when an issue runs and
            # slots are requested in ci order on every engine (v6/v7
            # deadlocked when the scheduler let a late chunk's issue grab a
            # slot ahead of an early one, or parked an issue with an
            # unsatisfied cap-wait inside the ACT stream). No dependency
            # ever lands ON a DMA instruction (that would wait for the
            # transfer, which cost v2/v3 ~35 us of ACT delay). ---
            u_ts = {ci: iop.tile([P, w, D], f32, name="u_t", bufs=u_bufs)
                    for ci in range(nch)}
            h2 = w // 2

            def is_scalar_chunk(ci):
                return ci <= 1 or (ci >= u_bufs and ci % 2 == 1)

            def issue_u(ci, piece=None, after=None):
                eng = nc.scalar if is_scalar_chunk(ci) else nc.sync
                if piece is None:
                    r0, r1 = 0, w
                else:
                    r0, r1 = piece * h2, (piece + 1) * h2
                inst = eng.dma_start(
                    u_ts[ci][:, r0:r1, :],
                    u_v[:, ci * w + r0 : ci * w + r1, :],
                )
                if after is not None:
                    add_dep_helper(
                        getattr(inst, "ins", inst), after, sync=True,
                        reason="u slot freed by this erf",
                    )
                return inst

            # t0: ONLY 3 issues on the scalar engine -- a 4th stalls on DMA
            # semaphore recycling and, being ahead of the ACT stream, blocked
            # warm+exp until ~26 us in v11. The sync engine may stall freely,
            # so it takes yg then chunks 2..u_bufs-1.
            issue_u(0, piece=0)
            issue_u(0, piece=1)
            issue_u(1)
            yg_ts = {}
            for h in range(n_half):
                yg_t = iop.tile([P, half, C], f32, name="yg_t", bufs=n_half)
                nc.sync.dma_start(yg_t[:], yg_v[:, h * half : (h + 1) * half, :])
                yg_ts[h] = yg_t
            for ci in range(2, u_bufs):
                issue_u(ci)
            # yhat via gpsimd SWDGE, casting i32 -> f32 in flight
            yh_f = constp.tile([P, rpp], f32)
            gp_ordered(nc.gpsimd.dma_start(yh_f[:], yh_v))

            # --- constants ---
            erf_bias = constp.tile([P, 1], f32)
            nc.vector.memset(erf_bias[:], 0.5 * SQ2)
            iota_f = constp.tile([P, 1, C], f32)
            gp_ordered(nc.gpsimd.iota(
                iota_f[:, 0, :], [[1, C]],
                channel_multiplier=0, allow_small_or_imprecise_dtypes=True,
            ))

            # warm the exp table while the first DMAs are in flight
            warm_act = constp.tile([1, 1], f32)
            nc.vector.memset(warm_act[:], 1.0)
            wa_o = constp.tile([1, 1], f32)
            sc_ordered(nc.scalar.activation(wa_o[:], warm_act[:], FT.Exp))

            # --- ACT phase 1: exp per yg piece, then one ln (shared table).
            # The DVE sumexp reduces MUST be emitted before the ln: the Tile
            # dep tracker orders accesses by emission, so a read emitted
            # before its writers gets no dependency (this was the v9/v10
            # NaN). The onehot for slab 0 is emitted first so it heads the
            # DVE chain. ---
            sume = constp.tile([P, rpp], f32)
            lse16 = constp.tile([P, rpp], f32)
            ohbig = constp.tile([P, rpp, C], bf16)

            def do_oneh(s):
                s0, s1 = s * slab, (s + 1) * slab
                dve_ordered(nc.vector.tensor_tensor(
                    ohbig[:, s0:s1, :],
                    yh_f[:, s0:s1].broadcast_to([P, slab, C]),
                    iota_f[:].broadcast_to([P, slab, C]),
                    ALU.is_equal,
                ))

            yge_ts = {}
            for h in range(n_half):
                yge = workp.tile([P, half, C], bf16, name="yge", bufs=yge_bufs)
                sc_ordered(nc.scalar.activation(yge[:], yg_ts[h][:], FT.Exp))
                yge_ts[h] = yge
                h0 = h * half
                dve_ordered(nc.vector.reduce_sum(
                    sume[:, h0 : h0 + half], yge[:], axis=AX.X
                ))
            sc_ordered(nc.scalar.activation(lse16[:], sume[:], FT.Ln))
            do_oneh(0)

            # --- ACT phase 2: the erf stream with chained scalar u issues ---
            work_ts = {}

            def do_erf(ci, parts=1):
                work_t = workp.tile([P, w, W_COLS], bf16, name="work_t", bufs=w_bufs)
                u_t = u_ts.pop(ci)
                step = w // parts
                for k in range(parts):
                    r0, r1 = k * step, (k + 1) * step
                    sc_ordered(
                        nc.scalar.activation(
                            work_t[:, r0:r1, 0:D], u_t[:, r0:r1, :], FT.Erf,
                            bias=erf_bias[:], scale=SQ2,
                        )
                    )
                work_ts[ci] = work_t

            for ci in range(nch):
                do_erf(ci, parts=2 if ci in (0, nch - 1) else 1)
                nxt = ci + u_bufs
                if nxt < nch:
                    issue_u(nxt, after=sc_box[0])

            # --- DVE chain continues: side cols c0..c3, oneh s1..s3, side
            # cols c4..c15, accS ---
            def do_side(ci):
                r0 = ci * w
                h = ci // (half // w)
                hr0 = (ci % (half // w)) * w
                pg_t = workp.tile([P, w, C], bf16, name="pg_t", bufs=2)
                dve_ordered(nc.vector.tensor_tensor(
                    pg_t[:], ohbig[:, r0 : r0 + w, :],
                    yg_ts[h][:, hr0 : hr0 + w, :], ALU.mult,
                ))
                with nc.allow_low_precision(reason="picked row has 1 nonzero"):
                    dve_ordered(nc.vector.reduce_sum(
                        work_ts[ci][:, :, COL_PICK], pg_t[:], axis=AX.X
                    ))
                dve_ordered(nc.vector.memset(work_ts[ci][:, :, COL_ONES], 1.0))
                r0 = ci * w
                dve_ordered(nc.vector.tensor_copy(
                    work_ts[ci][:, :, COL_LSE], lse16[:, r0 : r0 + w]
                ))

            chunks_per_slab = slab // w
            for ci in range(chunks_per_slab):
                do_side(ci)
            for s in range(1, n_slab):
                do_oneh(s)
            for ci in range(chunks_per_slab, nch):
                do_side(ci)

            # --- PE segment accumulation ---
            ps = psump.tile([C, W_COLS], f32)
            for ci in range(nch):
                r0 = ci * w
                work_t = work_ts.pop(ci)
                for g in range(w):
                    first = ci == 0 and g == 0
                    last = ci == nch - 1 and g == w - 1
                    nc.tensor.matmul(
                        ps[:], ohbig[:, r0 + g, :], work_t[:, g, :],
                        start=first, stop=last,
                    )

            # --- emit the raw accumulator; host finishes ---
            accS = constp.tile([C, W_OUT], f32)
            dve_ordered(nc.vector.tensor_copy(accS[:], ps[:]))
            nc.sync.dma_start(out_d[:], accS[:])

    nc.compile()
    return nc


_NC_CACHE = {}


def _get_nc(**kw):
    key = tuple(sorted(kw.items()))
    if key not in _NC_CACHE:
        _NC_CACHE[key] = build(**kw)
    return _NC_CACHE[key]


def make_in_maps(yhat, yg, u_zg, rows_per_core=ROWS_PER_CORE):
    yhat = np.ascontiguousarray(np.asarray(yhat).astype(np.int32))
    yg = np.ascontiguousarray(np.asarray(yg, dtype=np.float32))
    u_zg = np.ascontiguousarray(np.asarray(u_zg, dtype=np.float32))
    n = yhat.shape[0]
    assert n == rows_per_core * N_CORES
    in_maps = []
    for i in range(N_CORES):
        s = slice(i * rows_per_core, (i + 1) * rows_per_core)
        in_maps.append({"yhat": yhat[s], "yg": yg[s], "u_zg": u_zg[s]})
    return in_maps


def _finish(acc_sum, lmbd):
    """acc_sum: [C, W_OUT] f64 summed over cores. ~50 flops in numpy."""
    seg_erf = acc_sum[:, 0:D].sum(axis=1)
    seg_pick = acc_sum[:, COL_PICK]
    cnt = acc_sum[:, COL_ONES]
    seg_lse = acc_sum[:, COL_LSE]
    present = cnt > 0
    denom = np.where(present, cnt, 1.0)
    seg_reg = 0.5 * D * cnt + 0.5 * seg_erf
    reg_c = seg_reg / (denom * D)
    aux_c = (seg_lse - seg_pick) / denom
    n_unique = present.sum()
    reg = np.where(present, reg_c, 0.0).sum() / n_unique
    aux = np.where(present, aux_c, 0.0).sum() / n_unique
    return np.float32(aux + float(lmbd) * reg)


def run(yhat, yg, u_zg, lmbd, trace=False, rows_per_core=ROWS_PER_CORE, **kw):
    from concourse import bass_utils

    nc = _get_nc(rows_per_core=rows_per_core, **kw)
    in_maps = make_in_maps(yhat, yg, u_zg, rows_per_core)
    res = bass_utils.run_bass_kernel_spmd(
        nc, in_maps, core_ids=list(range(N_CORES)), trace=trace
    )
    acc = np.zeros((C, W_OUT), dtype=np.float64)
    for r in res.results:
        acc += np.asarray(r["out"], dtype=np.float64)
    val = _finish(acc, lmbd)
    return val, res


def kernel(yhat, yg, u_zg, lmbd):
    val, _ = run(yhat, yg, u_zg, lmbd)
    return np.asarray(val, dtype=np.float32).reshape(())


# revision 30
# speedup vs baseline: 1.6342x; 1.0711x over previous
"""Trainium2 Bass kernel for nn_AuxLoss (aux CE loss + erf regularizer, segment-
mean over K=10 classes), data-parallel over 8 NeuronCores.

Math (per reference):
  f(u)      = 0.5 - 0.5*erf((-0.5 - u)/(sigma*sqrt2)) = 0.5 + 0.5*erf(sqrt2*u + sqrt2/2)
  row_reg_n = sum_d f(u[n,d])
  row_ce_n  = logsumexp(yg[n,:]) - yg[n, yhat[n]]
  per-class means over rows with yhat==k, averaged over present classes:
  out = aux + lmbd * reg

v10 design. Measured facts: each HWDGE queue sustains ~270-350 GB/s and the
pair ~530+ GB/s, so the wire is not the limit -- the single ACT engine is
(erf 16 x 3.71 us at work-stride ~66, exp 8 x 1.63 us, ln 1.2 us). The
kernel is built around an uninterrupted ACT stream:
  - ACT chain: warm-exp, exp x8 (one per 128-row yg piece, gated only on its
    own DMA), ln, then erf c0..c15 back-to-back (first and last chunks in 2
    pieces: the first so erf starts as soon as 1 MB has landed, the last to
    shorten the PE tail). exp and ln share one ACT table set, so the whole
    kernel pays exactly 3 table loads (exp prewarmed at t~0, ln, erf).
    Because ln completes before the first erf, lse can ride the work tile
    like every other per-row quantity -- no separate segment path at all.
  - Scheduler-proofing (v5-v8 deadlocked or mis-scheduled): u DMA issues for
    chunks >= u_bufs carry a branch dependency on the erf whose completion
    frees their pool slot, so the counting cap-gate is satisfied by
    construction and slots are requested in chunk order; every DVE
    instruction sits on one explicit chain in dependency-forward order. No
    dependency ever lands ON a DMA instruction (that waits for the
    *transfer*, which cost v2/v3 ~35 us of ACT delay).
  - DMA: scalar HWDGE queue carries u0 (2 pieces), u1, u2, then odd chunks;
    sync HWDGE queue carries the 8 yg pieces first (full-resident pool, no
    WARs) then even u chunks. Tail chunks alternate queues so per-queue
    transfer pace (5.9 us) beats the 2-chunks-per-queue erf pace (7.4 us).
    yhat rides the gpsimd SWDGE queue, casting i32->f32 in flight.
  - NO on-device collective: each core emits its raw [10, 67] f32 segment
    accumulator; the host sums the 8 cores and does the ~50-flop finish in
    numpy (the legitimate unshard step). v1's AllGather cost a ~44 us tail.
  - per 64-row chunk one bf16 work tile [128, 64, 67]:
      cols 0:64  erf(sqrt2*u + sqrt2/2)    (ACT, strided out; the 0.5+0.5*
                 affine is folded into the host fixup)
      col  64    picked = sum_c onehot*yg  (DVE row-reduce of onehot*raw yg
                                            straight into the work column)
      col  65    ones                      (counts)
      col  66    lse                       (DVE bf16 copy from the ln output)
  - PE: per 128-row group ldweights(onehot[128,10]) + matmul(work[128,67]),
    accumulating PSUM [10,67] over all 1024 groups; onehot lives in one big
    bf16 tile computed early from yhat via iota-compare.
"""

import math
import sys

if "/opt/trn_rl_repo" not in sys.path:
    sys.path.insert(0, "/opt/trn_rl_repo")

import numpy as np

N_CORES = 8
N_FULL = 1048576
C = 10
D = 64
P = 128
ROWS_PER_CORE = N_FULL // N_CORES  # 131072
SQ2 = math.sqrt(2.0)
W_COLS = D + 3  # erf block | picked | ones | lse
W_OUT = W_COLS

COL_PICK = D
COL_ONES = D + 1
COL_LSE = D + 2


def build(rows_per_core=ROWS_PER_CORE, w=64, n_slab=4, n_half=8, u_bufs=6,
          w_bufs=3, yge_bufs=2):
    from concourse import bacc, mybir, tile

    f32 = mybir.dt.float32
    bf16 = mybir.dt.bfloat16
    i32 = mybir.dt.int32
    FT = mybir.ActivationFunctionType
    ALU = mybir.AluOpType
    AX = mybir.AxisListType

    rpp = rows_per_core // P  # rows per partition (1024)
    assert rpp * P == rows_per_core
    nch = rpp // w  # chunks (16)
    assert nch * w == rpp
    slab = rpp // n_slab  # lse slab rows (256)
    assert slab * n_slab == rpp
    half = rpp // n_half  # yg piece rows (128)
    assert half * n_half == rpp
    assert w <= half and half % w == 0

    nc = bacc.Bacc("TRN2", target_bir_lowering=False, debug=False, num_devices=N_CORES)

    yh_d = nc.dram_tensor("yhat", [rows_per_core], i32, kind="ExternalInput")
    yg_d = nc.dram_tensor("yg", [rows_per_core, C], f32, kind="ExternalInput")
    u_d = nc.dram_tensor("u_zg", [rows_per_core, D], f32, kind="ExternalInput")
    out_d = nc.dram_tensor("out", [C, W_OUT], f32, kind="ExternalOutput")

    u_v = u_d[:].rearrange("(p r) d -> p r d", p=P)
    yg_v = yg_d[:].rearrange("(p r) c -> p r c", p=P)
    yh_v = yh_d[:].rearrange("(p r) -> p r", p=P)

    from concourse.tile_rust import add_dep_helper

    def mk_chain(box, reason):
        def link(inst):
            raw = getattr(inst, "ins", inst)
            if box[0] is not None:
                add_dep_helper(raw, box[0], sync=True, reason=reason)
            box[0] = raw
            return inst
        return link

    sc_box = [None]
    sc_ordered = mk_chain(sc_box, "act order")
    dve_ordered = mk_chain([None], "dve order")
    gp_ordered = mk_chain([None], "gpsimd order")

    with tile.TileContext(nc) as tc:
        with (
            tc.tile_pool(name="const", bufs=1) as constp,
            tc.tile_pool(name="io", bufs=1) as iop,
            tc.tile_pool(name="work", bufs=1) as workp,
            tc.tile_pool(name="psum", bufs=1, space="PSUM") as psump,
        ):
            # --- u tiles created in ci order. Chunks 0..3 use fresh pool
            # slots and are issued at t0; every chunk k >= u_bufs is issued
            # with a branch dependency on erf(k - u_bufs) -- the compute
            # instruction whose completion frees its slot -- so the counting
            # cap-gate is always already satisfied when an issue runs and
            # slots are requested in ci order on every engine (v6/v7
            # deadlocked when the scheduler let a late chunk's issue grab a
            # slot ahead of an early one, or parked an issue with an
            # unsatisfied cap-wait inside the ACT stream). No dependency
            # ever lands ON a DMA instruction (that would wait for the
            # transfer, which cost v2/v3 ~35 us of ACT delay). ---
            u_ts = {ci: iop.tile([P, w, D], f32, name="u_t", bufs=u_bufs)
                    for ci in range(nch)}
            h2 = w // 2

            def is_scalar_chunk(ci):
                return ci <= 1 or ci in (4, 5) or (ci >= u_bufs and ci % 2 == 1)

            def issue_u(ci, piece=None, after=None):
                eng = nc.scalar if is_scalar_chunk(ci) else nc.sync
                if piece is None:
                    r0, r1 = 0, w
                else:
                    r0, r1 = piece * h2, (piece + 1) * h2
                inst = eng.dma_start(
                    u_ts[ci][:, r0:r1, :],
                    u_v[:, ci * w + r0 : ci * w + r1, :],
                )
                if after is not None:
                    add_dep_helper(
                        getattr(inst, "ins", inst), after, sync=True,
                        reason="u slot freed by this erf",
                    )
                return inst

            # t0: ONLY 3 issues on the scalar engine -- a 4th stalls on DMA
            # semaphore recycling and, being ahead of the ACT stream, blocked
            # warm+exp until ~26 us in v11. The sync engine may stall freely,
            # so it takes yg then chunks 2..u_bufs-1.
            issue_u(0, piece=0)
            issue_u(0, piece=1)
            issue_u(1)
            yg_ts = {}
            for h in range(n_half):
                yg_t = iop.tile([P, half, C], f32, name="yg_t", bufs=n_half)
                nc.sync.dma_start(yg_t[:], yg_v[:, h * half : (h + 1) * half, :])
                yg_ts[h] = yg_t
            for ci in (2, 3):
                issue_u(ci)
            # yhat via gpsimd SWDGE, casting i32 -> f32 in flight
            yh_f = constp.tile([P, rpp], f32)
            gp_ordered(nc.gpsimd.dma_start(yh_f[:], yh_v))

            # --- constants ---
            erf_bias = constp.tile([P, 1], f32)
            nc.vector.memset(erf_bias[:], 0.5 * SQ2)
            iota_f = constp.tile([P, 1, C], f32)
            gp_ordered(nc.gpsimd.iota(
                iota_f[:, 0, :], [[1, C]],
                channel_multiplier=0, allow_small_or_imprecise_dtypes=True,
            ))

            # warm the exp table while the first DMAs are in flight
            warm_act = constp.tile([1, 1], f32)
            nc.vector.memset(warm_act[:], 1.0)
            wa_o = constp.tile([1, 1], f32)
            sc_ordered(nc.scalar.activation(wa_o[:], warm_act[:], FT.Exp))

            # --- ACT phase 1: exp per yg piece, then one ln (shared table).
            # The DVE sumexp reduces MUST be emitted before the ln: the Tile
            # dep tracker orders accesses by emission, so a read emitted
            # before its writers gets no dependency (this was the v9/v10
            # NaN). The onehot for slab 0 is emitted first so it heads the
            # DVE chain. ---
            sume = constp.tile([P, rpp], f32)
            lse16 = constp.tile([P, rpp], f32)
            ohbig = constp.tile([P, rpp, C], bf16)

            def do_oneh(s):
                s0, s1 = s * slab, (s + 1) * slab
                dve_ordered(nc.vector.tensor_tensor(
                    ohbig[:, s0:s1, :],
                    yh_f[:, s0:s1].broadcast_to([P, slab, C]),
                    iota_f[:].broadcast_to([P, slab, C]),
                    ALU.is_equal,
                ))

            yge_ts = {}
            exp_insts = []
            for h in range(n_half):
                yge = workp.tile([P, half, C], bf16, name="yge", bufs=yge_bufs)
                ei = sc_ordered(nc.scalar.activation(yge[:], yg_ts[h][:], FT.Exp))
                exp_insts.append(getattr(ei, "ins", ei))
                yge_ts[h] = yge
                h0 = h * half
                dve_ordered(nc.vector.reduce_sum(
                    sume[:, h0 : h0 + half], yge[:], axis=AX.X
                ))
            issue_u(4, after=exp_insts[1])
            issue_u(5, after=exp_insts[3])
            sc_ordered(nc.scalar.activation(lse16[:], sume[:], FT.Ln))
            do_oneh(0)

            # --- ACT phase 2: the erf stream with chained scalar u issues ---
            work_ts = {}

            def do_erf(ci, parts=1):
                work_t = workp.tile([P, w, W_COLS], bf16, name="work_t", bufs=w_bufs)
                u_t = u_ts.pop(ci)
                step = w // parts
                for k in range(parts):
                    r0, r1 = k * step, (k + 1) * step
                    sc_ordered(
                        nc.scalar.activation(
                            work_t[:, r0:r1, 0:D], u_t[:, r0:r1, :], FT.Erf,
                            bias=erf_bias[:], scale=SQ2,
                        )
                    )
                work_ts[ci] = work_t

            for ci in range(nch):
                do_erf(ci, parts=4 if ci == nch - 1 else (2 if ci == 0 else 1))
                nxt = ci + u_bufs
                if nxt < nch:
                    issue_u(nxt, after=sc_box[0])

            # --- DVE chain continues: side cols c0..c3, oneh s1..s3, side
            # cols c4..c15, accS ---
            def do_side(ci):
                r0 = ci * w
                h = ci // (half // w)
                hr0 = (ci % (half // w)) * w
                pg_t = workp.tile([P, w, C], bf16, name="pg_t", bufs=1)
                dve_ordered(nc.vector.tensor_tensor(
                    pg_t[:], ohbig[:, r0 : r0 + w, :],
                    yg_ts[h][:, hr0 : hr0 + w, :], ALU.mult,
                ))
                with nc.allow_low_precision(reason="picked row has 1 nonzero"):
                    dve_ordered(nc.vector.reduce_sum(
                        work_ts[ci][:, :, COL_PICK], pg_t[:], axis=AX.X
                    ))
                dve_ordered(nc.vector.memset(work_ts[ci][:, :, COL_ONES], 1.0))
                r0 = ci * w
                dve_ordered(nc.vector.tensor_copy(
                    work_ts[ci][:, :, COL_LSE], lse16[:, r0 : r0 + w]
                ))

            chunks_per_slab = slab // w
            for ci in range(chunks_per_slab):
                do_side(ci)
            for s in range(1, n_slab):
                do_oneh(s)
            for ci in range(chunks_per_slab, nch):
                do_side(ci)

            # --- PE segment accumulation ---
            ps = psump.tile([C, W_COLS], f32)
            for ci in range(nch):
                r0 = ci * w
                work_t = work_ts.pop(ci)
                for g in range(w):
                    first = ci == 0 and g == 0
                    last = ci == nch - 1 and g == w - 1
                    nc.tensor.matmul(
                        ps[:], ohbig[:, r0 + g, :], work_t[:, g, :],
                        start=first, stop=last,
                    )

            # --- emit the raw accumulator; host finishes ---
            accS = constp.tile([C, W_OUT], f32)
            dve_ordered(nc.vector.tensor_copy(accS[:], ps[:]))
            nc.sync.dma_start(out_d[:], accS[:])

    nc.compile()
    return nc


_NC_CACHE = {}


def _get_nc(**kw):
    key = tuple(sorted(kw.items()))
    if key not in _NC_CACHE:
        _NC_CACHE[key] = build(**kw)
    return _NC_CACHE[key]


def make_in_maps(yhat, yg, u_zg, rows_per_core=ROWS_PER_CORE):
    yhat = np.ascontiguousarray(np.asarray(yhat).astype(np.int32))
    yg = np.ascontiguousarray(np.asarray(yg, dtype=np.float32))
    u_zg = np.ascontiguousarray(np.asarray(u_zg, dtype=np.float32))
    n = yhat.shape[0]
    assert n == rows_per_core * N_CORES
    in_maps = []
    for i in range(N_CORES):
        s = slice(i * rows_per_core, (i + 1) * rows_per_core)
        in_maps.append({"yhat": yhat[s], "yg": yg[s], "u_zg": u_zg[s]})
    return in_maps


def _finish(acc_sum, lmbd):
    """acc_sum: [C, W_OUT] f64 summed over cores. ~50 flops in numpy."""
    seg_erf = acc_sum[:, 0:D].sum(axis=1)
    seg_pick = acc_sum[:, COL_PICK]
    cnt = acc_sum[:, COL_ONES]
    seg_lse = acc_sum[:, COL_LSE]
    present = cnt > 0
    denom = np.where(present, cnt, 1.0)
    seg_reg = 0.5 * D * cnt + 0.5 * seg_erf
    reg_c = seg_reg / (denom * D)
    aux_c = (seg_lse - seg_pick) / denom
    n_unique = present.sum()
    reg = np.where(present, reg_c, 0.0).sum() / n_unique
    aux = np.where(present, aux_c, 0.0).sum() / n_unique
    return np.float32(aux + float(lmbd) * reg)


def run(yhat, yg, u_zg, lmbd, trace=False, rows_per_core=ROWS_PER_CORE, **kw):
    from concourse import bass_utils

    nc = _get_nc(rows_per_core=rows_per_core, **kw)
    in_maps = make_in_maps(yhat, yg, u_zg, rows_per_core)
    res = bass_utils.run_bass_kernel_spmd(
        nc, in_maps, core_ids=list(range(N_CORES)), trace=trace
    )
    acc = np.zeros((C, W_OUT), dtype=np.float64)
    for r in res.results:
        acc += np.asarray(r["out"], dtype=np.float64)
    val = _finish(acc, lmbd)
    return val, res


def kernel(yhat, yg, u_zg, lmbd):
    val, _ = run(yhat, yg, u_zg, lmbd)
    return np.asarray(val, dtype=np.float32).reshape(())
